# revision 24
# baseline (speedup 1.0000x reference)
"""GCN (2-layer graph conv, DGL norm='both') on 8 Trainium2 NeuronCores.

Strategy
--------
- Nodes are bin-packed into 8*BINS bins of 128 (balanced in-degree sums).
  Each core owns BINS bins (dst-partitioning of the graph).
- Layer GEMMs are transform-first: z = (x * norm_src) @ W, sharded by node
  owner, bf16 inputs / fp32 PSUM accumulate.
- z (bf16) is AllGather'd (in row-chunks, overlapped with the producing
  GEMMs) to every core's HBM; per-edge messages are pulled with dma_gather
  (int16 indices -> two overlapping row-windows A/B of the table so every
  row index fits in int16). Gather calls cover PAIRS of bins (2048 idxs)
  and rotate across the 4 SWDGE queues so descriptor generation runs on
  all four Q7 core-pairs concurrently.
- Segment-sum over edges = selection-matrix matmul: S[p, c] = (dstloc[p]==c)
  built on DVE via is_equal, accumulated per 128-dst bin in PSUM.
- The layer-2 input GEMM (z2 = relu(h1)*ns @ W2) is fused into the layer-1
  aggregation epilogue so the z2 AllGather chunks stream out while later
  bins still aggregate.
- Same edge structure (indices, S data) is reused for both layers.
"""

import math
import os
import threading

import numpy as np

import concourse.bacc as bacc
import concourse.bass as bass
import concourse.mybir as mybir
import concourse.tile as tile

P = 128
F32 = mybir.dt.float32
BF16 = mybir.dt.bfloat16
I16 = mybir.dt.int16


class Cfg:
    def __init__(self, N, E, F0, F1, F2, NC, BINS, T_A, T_B, WIN, GROUP):
        self.N = N            # real node count
        self.E = E            # edge count
        self.F0, self.F1, self.F2 = F0, F1, F2
        self.F2P = 128        # z2 rows padded to 128 cols (gather elem >= 256B)
        self.NC = NC          # cores
        self.BINS = BINS      # dst bins (of 128 nodes) per core
        self.NPC = BINS * P   # padded nodes per core
        self.NPOS = NC * self.NPC
        self.T_A = T_A        # msg tiles per bin from window A
        self.T_B = T_B        # msg tiles per bin from window B
        self.WIN = WIN        # window size (int16 index reach)
        self.B_BASE = max(0, self.NPOS - WIN)
        self.GROUP = GROUP    # bins per gather call
        assert self.NPOS <= WIN + self.B_BASE  # windows cover all rows
        assert self.B_BASE < WIN               # overlap (flex) region exists


CFG_FULL = Cfg(N=50000, E=800000, F0=512, F1=256, F2=64, NC=8,
               BINS=49, T_A=8, T_B=8, WIN=32768,
               GROUP=int(os.environ.get("GCN_GROUP", "1")))


# --------------------------------------------------------------------------
# Host-side graph preprocessing
# --------------------------------------------------------------------------

def _preprocess(cfg, src, dst):
    """Assign nodes to (core, bin, slot) positions and build padded edge
    streams. Returns a dict of per-core numpy arrays plus the node->position
    permutation."""
    N, NC, BINS = cfg.N, cfg.NC, cfg.BINS
    NBINS = NC * BINS
    deg_in = np.bincount(dst, minlength=N).astype(np.int64)
    deg_out = np.bincount(src, minlength=N).astype(np.int64)

    # --- bin-pack nodes by in-degree: balanced sums, <=128 nodes per bin ---
    order = np.argsort(-deg_in, kind="stable")
    bin_sum = np.zeros(NBINS, dtype=np.int64)
    bin_cnt = np.zeros(NBINS, dtype=np.int64)
    bin_nodes = [[] for _ in range(NBINS)]
    import heapq
    heap = [(0, 0, b) for b in range(NBINS)]  # (sum, cnt, bin)
    heapq.heapify(heap)
    for n in order:
        while True:
            s, c, b = heapq.heappop(heap)
            if c < P and s == bin_sum[b] and c == bin_cnt[b]:
                break
        bin_nodes[b].append(n)
        bin_sum[b] += deg_in[n]
        bin_cnt[b] += 1
        if bin_cnt[b] < P:
            heapq.heappush(heap, (int(bin_sum[b]), int(bin_cnt[b]), b))
    capT = (cfg.T_A + cfg.T_B) * P
    assert bin_sum.max() <= capT, (bin_sum.max(), capT)

    # --- positions ---
    pos = np.full(cfg.NPOS, -1, dtype=np.int64)   # position -> node (-1 pad)
    node_pos = np.zeros(N, dtype=np.int64)        # node -> position
    for b in range(NBINS):
        base = b * P
        for s, n in enumerate(bin_nodes[b]):
            pos[base + s] = n
            node_pos[n] = base + s

    # --- norms (match reference._sym_norms) ---
    norm_src = np.where(deg_out > 0, 1.0 / np.sqrt(np.maximum(deg_out, 1)),
                        1.0).astype(np.float32)
    norm_dst = np.where(deg_in > 0, 1.0 / np.sqrt(np.maximum(deg_in, 1)),
                        1.0).astype(np.float32)

    # --- edge streams per bin ---
    psrc = node_pos[src]              # gather position of each edge's source
    pdst = node_pos[dst]
    ebin = pdst // P                  # destination bin of each edge
    eslot = pdst % P                  # dst slot within bin
    capA, capB = cfg.T_A * P, cfg.T_B * P

    # per-bin edge lists
    idx_sort = np.argsort(ebin, kind="stable")
    ebin_s = ebin[idx_sort]
    bounds = np.searchsorted(ebin_s, np.arange(NBINS + 1))

    idxA = np.zeros((NBINS, capA), dtype=np.int16)
    idxB = np.zeros((NBINS, capB), dtype=np.int16)
    dlA = np.full((NBINS, capA), -1.0, dtype=np.float32)
    dlB = np.full((NBINS, capB), -1.0, dtype=np.float32)
    for b in range(NBINS):
        eids = idx_sort[bounds[b]:bounds[b + 1]]
        ps = psrc[eids]
        sl = eslot[eids]
        a_only = ps < cfg.B_BASE
        b_only = ps >= cfg.WIN
        flex = ~a_only & ~b_only
        nA, nB, nF = a_only.sum(), b_only.sum(), flex.sum()
        assert nA <= capA and nB <= capB, (b, nA, nB)
        flexA = min(nF, capA - nA)
        assert flexA >= 0, (b, nA, capA)
        assert nB + (nF - flexA) <= capB, (b, nA, nB, nF)
        fidx = np.nonzero(flex)[0]
        a_sel = np.concatenate([np.nonzero(a_only)[0], fidx[:flexA]])
        b_sel = np.concatenate([np.nonzero(b_only)[0], fidx[flexA:]])
        idxA[b, :len(a_sel)] = ps[a_sel]
        dlA[b, :len(a_sel)] = sl[a_sel]
        idxB[b, :len(b_sel)] = ps[b_sel] - cfg.B_BASE
        dlB[b, :len(b_sel)] = sl[b_sel]

    return dict(node_pos=node_pos, pos=pos, norm_src=norm_src,
                norm_dst=norm_dst, idxA=idxA, idxB=idxB, dlA=dlA, dlB=dlB)


def _wrap_idx(cfg, idx_bins, Tn):
    """Build the dma_gather index SBUF image for one core & stream:
    [128, total_cols] int16. Within a call, logical index j lives at
    [j%16, j//16]; the 16-row pattern is replicated to all 128 partitions.
    Per-bin images concatenated horizontally are identical to the image of
    any contiguous multi-bin call, so calls may span several bins."""
    cols = []
    for v in idx_bins:                                # one block per bin
        arr = np.zeros((16, len(v) // 16), dtype=np.int16)
        j = np.arange(len(v))
        arr[j % 16, j // 16] = v
        cols.append(arr)
    full = np.concatenate(cols, axis=1)
    return np.tile(full, (8, 1))                      # replicate to 128 parts


def _core_inputs(cfg, core, pre, feat, W1, b1, W2, b2):
    """Build the input-tensor dict for one core."""
    NPC, BINS = cfg.NPC, cfg.BINS
    base = core * NPC
    pslice = pre["pos"][base:base + NPC]              # node id or -1 per slot
    valid = pslice >= 0
    featc = np.zeros((NPC, cfg.F0), dtype=np.float32)
    featc[valid] = feat[pslice[valid]]
    ns = np.ones(NPC, dtype=np.float32)
    nd = np.ones(NPC, dtype=np.float32)
    ns[valid] = pre["norm_src"][pslice[valid]]
    nd[valid] = pre["norm_dst"][pslice[valid]]

    iota_cols = cfg.GROUP * max(cfg.T_A, cfg.T_B) * P
    bsl = slice(core * BINS, (core + 1) * BINS)
    d = {
        "featT": np.ascontiguousarray(featc.T).astype(mybir.dt.np(BF16)),
        "W1": W1.astype(mybir.dt.np(BF16)),
        "W2": W2.astype(mybir.dt.np(BF16)),
        "b1_bc": np.broadcast_to(b1, (P, cfg.F1)).copy().astype(np.float32),
        "b2_bc": np.broadcast_to(b2, (P, cfg.F2)).copy().astype(np.float32),
        "ns_cols": np.ascontiguousarray(ns.reshape(BINS, P).T),
        "nd_cols": np.ascontiguousarray(nd.reshape(BINS, P).T),
        "ndns_cols": np.ascontiguousarray((ns * nd).reshape(BINS, P).T),
        "iota_big": np.broadcast_to(
            np.tile(np.arange(P, dtype=np.float32),
                    iota_cols // P),
            (P, iota_cols)).copy().astype(mybir.dt.np(BF16)),
        "idxA": _wrap_idx(cfg, pre["idxA"][bsl], cfg.T_A),
        "idxB": _wrap_idx(cfg, pre["idxB"][bsl], cfg.T_B),
        "dlA": np.ascontiguousarray(
            pre["dlA"][bsl].reshape(BINS * cfg.T_A, P).T
        ).astype(mybir.dt.np(BF16)),
        "dlB": np.ascontiguousarray(
            pre["dlB"][bsl].reshape(BINS * cfg.T_B, P).T
        ).astype(mybir.dt.np(BF16)),
        "identity": np.eye(P, dtype=np.float32).astype(mybir.dt.np(BF16)),
    }
    return d


# --------------------------------------------------------------------------
# Device program
# --------------------------------------------------------------------------

def build_gcn(tc, outs, ins, cfg, b1_zero, b2_zero, stop_after=99):
    nc = tc.nc
    BINS, GROUP = cfg.BINS, cfg.GROUP
    T_A, T_B = cfg.T_A, cfg.T_B
    F0, F1, F2, F2P = cfg.F0, cfg.F1, cfg.F2, cfg.F2P
    NPC, NPOS = cfg.NPC, cfg.NPOS
    K0, K1 = F0 // P, F1 // P
    out_ap = outs["out"]
    NCH = 4  # allgather chunks per layer
    chunk_at = [round(i * BINS / NCH) for i in range(1, NCH + 1)]
    pairs = [list(range(b, min(b + GROUP, BINS)))
             for b in range(0, BINS, GROUP)]

    import contextlib
    ctx = contextlib.ExitStack()
    with ctx:

        def _bail():
            with tc.tile_pool(name="bailp", bufs=1) as bp:
                zt = bp.tile([P, cfg.F2], F32)
                nc.vector.memset(zt[:], 0.0)
                for b in range(BINS):
                    nc.sync.dma_start(out=out_ap[b * P:(b + 1) * P, :], in_=zt[:])

        constp = ctx.enter_context(tc.tile_pool(name="constp", bufs=1))
        residp = ctx.enter_context(tc.tile_pool(name="residp", bufs=1))
        dramp = ctx.enter_context(tc.tile_pool(name="dramp", bufs=1, space="DRAM"))

        # ---- resident constants ----
        idxA_sb = constp.tile([P, ins["idxA"].shape[1]], I16)
        idxB_sb = constp.tile([P, ins["idxB"].shape[1]], I16)
        dlA_sb = constp.tile([P, BINS * T_A], BF16)
        dlB_sb = constp.tile([P, BINS * T_B], BF16)
        iota_sb = constp.tile([P, GROUP * max(T_A, T_B) * P], BF16)
        ns_sb = constp.tile([P, BINS], F32)
        nd_sb = constp.tile([P, BINS], F32)
        ndns_sb = constp.tile([P, BINS], F32)
        b1_sb = constp.tile([P, F1], F32)
        b2_sb = constp.tile([P, F2], F32)
        ident_sb = constp.tile([P, P], BF16)
        w1_sb = constp.tile([P, K0 * F1], BF16)   # k-chunk c at cols [c*F1, ...)
        w2_sb = constp.tile([P, K1 * F2], BF16)
        for name, t in [("idxA", idxA_sb), ("idxB", idxB_sb), ("dlA", dlA_sb),
                        ("dlB", dlB_sb), ("iota_big", iota_sb),
                        ("ns_cols", ns_sb), ("nd_cols", nd_sb),
                        ("ndns_cols", ndns_sb), ("b1_bc", b1_sb),
                        ("b2_bc", b2_sb), ("identity", ident_sb)]:
            nc.sync.dma_start(out=t[:], in_=ins[name][:])
        for c in range(K0):
            nc.sync.dma_start(out=w1_sb[:, c * F1:(c + 1) * F1],
                              in_=ins["W1"][c * P:(c + 1) * P, :])
        for c in range(K1):
            nc.sync.dma_start(out=w2_sb[:, c * F2:(c + 1) * F2],
                              in_=ins["W2"][c * P:(c + 1) * P, :])

        # residents for layer-2 input
        h1r = residp.tile([P, BINS * F1], BF16)      # relu(h1)*ns, bin-major
        h1rT0 = residp.tile([P, NPC], BF16)          # feat 0:128 transposed
        h1rT1 = residp.tile([P, NPC], BF16)          # feat 128:256

        # DRAM intermediates
        z1_loc = dramp.tile([NPC, F1], BF16)
        z1_full = dramp.tile([NPOS, F1], BF16, addr_space="Shared")
        z2_loc = dramp.tile([NPC, F2P], BF16)
        z2_full = dramp.tile([NPOS, F2P], BF16, addr_space="Shared")
        rg = [list(range(cfg.NC))]

        def allgather(z_loc, z_full):
            nc.gpsimd.collective_compute(
                "AllGather", mybir.AluOpType.bypass, replica_groups=rg,
                ins=[z_loc.opt()], outs=[z_full.opt()])

        # ---- phase 1: z1 = (x @ W1) * ns, allgathered in row chunks ----
        with tc.tile_pool(name="g1", bufs=1) as g1p, \
             tc.tile_pool(name="g1w", bufs=3) as g1w, \
             tc.tile_pool(name="g1ps", bufs=2, space="PSUM") as g1ps:
            featT_sb = g1p.tile([P, K0 * NPC], BF16)
            lo = 0
            for hi in chunk_at:  # chunk-major loads unblock early bins
                for c in range(K0):
                    nc.sync.dma_start(
                        out=featT_sb[:, c * NPC + lo * P:c * NPC + hi * P],
                        in_=ins["featT"][c * P:(c + 1) * P, lo * P:hi * P])
                lo = hi
            for b in range(BINS):
                ps = g1ps.tile([P, F1], F32, space="PSUM", tag="ps")
                for c in range(K0):
                    nc.tensor.matmul(
                        out=ps[:],
                        lhsT=featT_sb[:, c * NPC + b * P: c * NPC + (b + 1) * P],
                        rhs=w1_sb[:, c * F1:(c + 1) * F1],
                        start=(c == 0), stop=(c == K0 - 1))
                zt = g1w.tile([P, F1], BF16, tag="zt")
                nc.vector.tensor_scalar(
                    out=zt[:], in0=ps[:], scalar1=ns_sb[:, b:b + 1],
                    scalar2=None, op0=mybir.AluOpType.mult)
                nc.sync.dma_start(out=z1_loc[b * P:(b + 1) * P, :], in_=zt[:])

        if stop_after < 1:
            _bail()
            return
        allgather(z1_loc, z1_full)

        # Filler matmuls: the PE down-clocks to 1.2 GHz after ~3.4us idle.
        # Keep it busy through the allgather stalls so the aggregation
        # matmuls start (and stay) at 2.4 GHz.
        warmp = ctx.enter_context(tc.tile_pool(name="warmps", bufs=1,
                                               space="PSUM"))

        def warm_pe(n):
            for _ in range(n):
                wp = warmp.tile([P, 512], F32, space="PSUM", tag="wp")
                nc.tensor.matmul(out=wp[:], lhsT=ident_sb[:],
                                 rhs=iota_sb[:, :512], start=True, stop=True)

        warm_pe(60)
        if stop_after < 2:
            _bail()
            return

        qctr = [0]  # swdge queue rotation across all gather calls

        # ---- shared aggregation emitter (pairs of bins per gather) ----
        def aggregate(layer, z_tab, F, FV, epilogue, bin_done=None):
            """layer: 1|2, z_tab: DRAM [NPOS, F], FV: valid cols of gathered
            rows used as matmul rhs. epilogue(b, psum, epp) consumes a bin;
            bin_done(b) is called after each bin's epilogue."""
            with tc.tile_pool(name=f"msg{layer}", bufs=6) as msgp, \
                 tc.tile_pool(name=f"sp{layer}", bufs=4) as sp, \
                 tc.tile_pool(name=f"agg{layer}ps", bufs=4, space="PSUM") as aggps, \
                 tc.tile_pool(name=f"ep{layer}", bufs=3) as epp:
                for bs in pairs:
                    nb, b0 = len(bs), bs[0]
                    mA = msgp.tile([P, GROUP * T_A, F], BF16, tag="mA")
                    mB = msgp.tile([P, GROUP * T_B, F], BF16, tag="mB")
                    nc.gpsimd.dma_gather(
                        out_ap=mA[:, :nb * T_A, :], in_ap=z_tab[0:cfg.WIN, :],
                        idxs_ap=idxA_sb[:, b0 * T_A * 8:(b0 + nb) * T_A * 8],
                        num_idxs=nb * T_A * P, num_idxs_reg=nb * T_A * P,
                        elem_size=F, queue_num=qctr[0] % 4)
                    qctr[0] += 1
                    nc.gpsimd.dma_gather(
                        out_ap=mB[:, :nb * T_B, :],
                        in_ap=z_tab[cfg.B_BASE:NPOS, :],
                        idxs_ap=idxB_sb[:, b0 * T_B * 8:(b0 + nb) * T_B * 8],
                        num_idxs=nb * T_B * P, num_idxs_reg=nb * T_B * P,
                        elem_size=F, queue_num=qctr[0] % 4)
                    qctr[0] += 1
                    sA = sp.tile([P, GROUP * T_A * P], BF16, tag="sA")
                    sB = sp.tile([P, GROUP * T_B * P], BF16, tag="sB")
                    nc.vector.tensor_tensor(
                        out=sA[:, :nb * T_A * P].rearrange(
                            "p (t c) -> p t c", c=P),
                        in0=iota_sb[:, :nb * T_A * P].rearrange(
                            "p (t c) -> p t c", c=P),
                        in1=dlA_sb[:, b0 * T_A:(b0 + nb) * T_A, None]
                            .to_broadcast((P, nb * T_A, P)),
                        op=mybir.AluOpType.is_equal)
                    nc.vector.tensor_tensor(
                        out=sB[:, :nb * T_B * P].rearrange(
                            "p (t c) -> p t c", c=P),
                        in0=iota_sb[:, :nb * T_B * P].rearrange(
                            "p (t c) -> p t c", c=P),
                        in1=dlB_sb[:, b0 * T_B:(b0 + nb) * T_B, None]
                            .to_broadcast((P, nb * T_B, P)),
                        op=mybir.AluOpType.is_equal)
                    for i, b in enumerate(bs):
                        ps = aggps.tile([P, FV], F32, space="PSUM", tag="ps")
                        for t in range(T_A):
                            j = i * T_A + t
                            nc.tensor.matmul(
                                out=ps[:], lhsT=sA[:, j * P:(j + 1) * P],
                                rhs=mA[:, j, :FV],
                                start=(t == 0), stop=False)
                        for t in range(T_B):
                            j = i * T_B + t
                            nc.tensor.matmul(
                                out=ps[:], lhsT=sB[:, j * P:(j + 1) * P],
                                rhs=mB[:, j, :FV],
                                start=False, stop=(t == T_B - 1))
                        epilogue(b, ps, epp)
                        if bin_done is not None:
                            bin_done(b)

        # ---- phase 2: layer-1 aggregation -> h1r, fused z2 GEMM + AG2 ----
        with tc.tile_pool(name="trps", bufs=1, space="PSUM") as trps, \
             tc.tile_pool(name="g2w", bufs=3) as g2w, \
             tc.tile_pool(name="g2ps", bufs=2, space="PSUM") as g2ps:
            def epi1(b, ps, epp):
                hcols = h1r[:, b * F1:(b + 1) * F1]
                if b1_zero:
                    nc.vector.tensor_scalar(
                        out=hcols, in0=ps[:], scalar1=ndns_sb[:, b:b + 1],
                        scalar2=0.0, op0=mybir.AluOpType.mult,
                        op1=mybir.AluOpType.max)
                else:
                    tmp = epp.tile([P, F1], F32, tag="tmp")
                    nc.vector.scalar_tensor_tensor(
                        out=tmp[:], in0=ps[:], scalar=nd_sb[:, b:b + 1],
                        in1=b1_sb[:], op0=mybir.AluOpType.mult,
                        op1=mybir.AluOpType.add)
                    nc.vector.tensor_scalar(
                        out=hcols, in0=tmp[:], scalar1=0.0,
                        scalar2=ns_sb[:, b:b + 1], op0=mybir.AluOpType.max,
                        op1=mybir.AluOpType.mult)
                # transpose the two 128-col halves for the layer-2 GEMM
                for k, hT in ((0, h1rT0), (1, h1rT1)):
                    tp = trps.tile([P, P], BF16, space="PSUM", tag="tp")
                    nc.tensor.transpose(
                        out=tp[:], in_=h1r[:, b * F1 + k * P: b * F1 + (k + 1) * P],
                        identity=ident_sb[:])
                    nc.vector.tensor_copy(out=hT[:, b * P:(b + 1) * P], in_=tp[:])
                # fused phase 3: z2 = h1r @ W2 (padded to F2P cols)
                ps2 = g2ps.tile([P, F2], F32, space="PSUM", tag="ps2")
                nc.tensor.matmul(out=ps2[:], lhsT=h1rT0[:, b * P:(b + 1) * P],
                                 rhs=w2_sb[:, 0:F2], start=True, stop=False)
                nc.tensor.matmul(out=ps2[:], lhsT=h1rT1[:, b * P:(b + 1) * P],
                                 rhs=w2_sb[:, F2:2 * F2], start=False, stop=True)
                z2t = g2w.tile([P, F2P], BF16, tag="z2t")
                nc.vector.tensor_copy(out=z2t[:, :F2], in_=ps2[:])
                nc.vector.memset(z2t[:, F2:], 0.0)
                nc.sync.dma_start(out=z2_loc[b * P:(b + 1) * P, :], in_=z2t[:])

            aggregate(1, z1_full, F1, F1, epi1)

        if stop_after < 3:
            _bail()
            return
        allgather(z2_loc, z2_full)
        warm_pe(40)
        if stop_after < 4:
            _bail()
            return

        # ---- phase 4: layer-2 aggregation -> output ----
        def epi2(b, ps, epp):
            ot = epp.tile([P, F2], F32, tag="ot")
            if b2_zero:
                nc.vector.tensor_scalar(
                    out=ot[:], in0=ps[:, :F2], scalar1=nd_sb[:, b:b + 1],
                    scalar2=None, op0=mybir.AluOpType.mult)
            else:
                nc.vector.scalar_tensor_tensor(
                    out=ot[:], in0=ps[:, :F2], scalar=nd_sb[:, b:b + 1],
                    in1=b2_sb[:], op0=mybir.AluOpType.mult,
                    op1=mybir.AluOpType.add)
            nc.sync.dma_start(out=out_ap[b * P:(b + 1) * P, :], in_=ot[:])

        aggregate(2, z2_full, F2P, F2, epi2)


# --------------------------------------------------------------------------
# Entry point
# --------------------------------------------------------------------------

_cache = {}
_cache_lock = threading.Lock()


def _build_program(cfg, in_specs, b1_zero, b2_zero, stop_after=99, nbody=1):
    nc = bacc.Bacc("TRN2", target_bir_lowering=False, debug=False,
                   num_devices=cfg.NC, num_swdge_queues=4)
    in_aps = {
        name: nc.dram_tensor(name, list(a.shape), mybir.dt.from_np(a.dtype),
                             kind="ExternalInput").ap()
        for name, a in in_specs.items()
    }
    out_aps = {"out": nc.dram_tensor("out", [cfg.NPC, cfg.F2], F32,
                                     kind="ExternalOutput").ap()}
    with tile.TileContext(nc) as tc:
        for _ in range(nbody):
            build_gcn(tc, out_aps, in_aps, cfg, b1_zero, b2_zero,
                      stop_after=stop_after)
    nc.compile()
    return nc


def run_gcn(cfg, feat, src, dst, W1, b1, W2, b2, core_ids=None):
    from concourse import bass_utils

    pre = _preprocess(cfg, np.asarray(src), np.asarray(dst))
    in_maps = [
        _core_inputs(cfg, c, pre, np.asarray(feat, np.float32),
                     np.asarray(W1, np.float32), np.asarray(b1, np.float32),
                     np.asarray(W2, np.float32), np.asarray(b2, np.float32))
        for c in range(cfg.NC)
    ]
    b1_zero = bool(np.all(np.asarray(b1) == 0))
    b2_zero = bool(np.all(np.asarray(b2) == 0))
    stop_after = int(os.environ.get("GCN_STOP_AFTER", "99"))
    key = (id(cfg), b1_zero, b2_zero, stop_after)
    with _cache_lock:
        if key not in _cache:
            _cache[key] = _build_program(cfg, in_maps[0], b1_zero, b2_zero,
                                         stop_after=stop_after)
    nc = _cache[key]

    if core_ids is None:
        core_ids = list(range(cfg.NC))
    res = bass_utils.run_bass_kernel_spmd(
        nc, in_maps, core_ids=core_ids,
        trace=bool(int(os.environ.get("GCN_TRACE", "0"))))
    allout = np.concatenate([r["out"] for r in res.results], axis=0)
    out = allout[pre["node_pos"]].astype(np.float32)
    return out, res


def kernel(feat, src, dst, W1, b1, W2, b2):
    out, _ = run_gcn(CFG_FULL, feat, src, dst, W1, b1, W2, b2)
    return out


# revision 28
# speedup vs baseline: 1.0196x; 1.0196x over previous
"""GCN (2-layer graph conv, DGL norm='both') on 8 Trainium2 NeuronCores.

Strategy
--------
- Nodes are bin-packed into 8*BINS bins of 128 (balanced in-degree sums).
  Each core owns BINS bins (dst-partitioning of the graph).
- Layer GEMMs are transform-first: z = (x * norm_src) @ W, sharded by node
  owner, bf16 inputs / fp32 PSUM accumulate.
- z (bf16) is AllGather'd (in row-chunks, overlapped with the producing
  GEMMs) to every core's HBM; per-edge messages are pulled with dma_gather
  (int16 indices -> two overlapping row-windows A/B of the table so every
  row index fits in int16). Gather calls cover PAIRS of bins (2048 idxs)
  and rotate across the 4 SWDGE queues so descriptor generation runs on
  all four Q7 core-pairs concurrently.
- Segment-sum over edges = selection-matrix matmul: S[p, c] = (dstloc[p]==c)
  built on DVE via is_equal, accumulated per 128-dst bin in PSUM.
- The layer-2 input GEMM (z2 = relu(h1)*ns @ W2) is fused into the layer-1
  aggregation epilogue so the z2 AllGather chunks stream out while later
  bins still aggregate.
- Same edge structure (indices, S data) is reused for both layers.
"""

import math
import os
import threading

import numpy as np

import concourse.bacc as bacc
import concourse.bass as bass
import concourse.mybir as mybir
import concourse.tile as tile

P = 128
F32 = mybir.dt.float32
BF16 = mybir.dt.bfloat16
I16 = mybir.dt.int16


class Cfg:
    def __init__(self, N, E, F0, F1, F2, NC, BINS, T_A, T_B, WIN, GROUP):
        self.N = N            # real node count
        self.E = E            # edge count
        self.F0, self.F1, self.F2 = F0, F1, F2
        self.F2P = 128        # z2 rows padded to 128 cols (gather elem >= 256B)
        self.NC = NC          # cores
        self.BINS = BINS      # dst bins (of 128 nodes) per core
        self.NPC = BINS * P   # padded nodes per core
        self.NPOS = NC * self.NPC
        self.T_A = T_A        # msg tiles per bin from window A
        self.T_B = T_B        # msg tiles per bin from window B
        self.WIN = WIN        # window size (int16 index reach)
        self.B_BASE = max(0, self.NPOS - WIN)
        self.GROUP = GROUP    # bins per gather call
        assert self.NPOS <= WIN + self.B_BASE  # windows cover all rows
        assert self.B_BASE < WIN               # overlap (flex) region exists


CFG_FULL = Cfg(N=50000, E=800000, F0=512, F1=256, F2=64, NC=8,
               BINS=49, T_A=8, T_B=8, WIN=32768,
               GROUP=int(os.environ.get("GCN_GROUP", "1")))


# --------------------------------------------------------------------------
# Host-side graph preprocessing
# --------------------------------------------------------------------------

def _preprocess(cfg, src, dst):
    """Assign nodes to (core, bin, slot) positions and build padded edge
    streams. Returns a dict of per-core numpy arrays plus the node->position
    permutation."""
    N, NC, BINS = cfg.N, cfg.NC, cfg.BINS
    NBINS = NC * BINS
    deg_in = np.bincount(dst, minlength=N).astype(np.int64)
    deg_out = np.bincount(src, minlength=N).astype(np.int64)

    # --- bin-pack nodes by in-degree: balanced sums, <=128 nodes per bin ---
    order = np.argsort(-deg_in, kind="stable")
    bin_sum = np.zeros(NBINS, dtype=np.int64)
    bin_cnt = np.zeros(NBINS, dtype=np.int64)
    bin_nodes = [[] for _ in range(NBINS)]
    import heapq
    heap = [(0, 0, b) for b in range(NBINS)]  # (sum, cnt, bin)
    heapq.heapify(heap)
    for n in order:
        while True:
            s, c, b = heapq.heappop(heap)
            if c < P and s == bin_sum[b] and c == bin_cnt[b]:
                break
        bin_nodes[b].append(n)
        bin_sum[b] += deg_in[n]
        bin_cnt[b] += 1
        if bin_cnt[b] < P:
            heapq.heappush(heap, (int(bin_sum[b]), int(bin_cnt[b]), b))
    capT = (cfg.T_A + cfg.T_B) * P
    assert bin_sum.max() <= capT, (bin_sum.max(), capT)

    # --- positions ---
    pos = np.full(cfg.NPOS, -1, dtype=np.int64)   # position -> node (-1 pad)
    node_pos = np.zeros(N, dtype=np.int64)        # node -> position
    for b in range(NBINS):
        base = b * P
        for s, n in enumerate(bin_nodes[b]):
            pos[base + s] = n
            node_pos[n] = base + s

    # --- norms (match reference._sym_norms) ---
    norm_src = np.where(deg_out > 0, 1.0 / np.sqrt(np.maximum(deg_out, 1)),
                        1.0).astype(np.float32)
    norm_dst = np.where(deg_in > 0, 1.0 / np.sqrt(np.maximum(deg_in, 1)),
                        1.0).astype(np.float32)

    # --- edge streams per bin ---
    psrc = node_pos[src]              # gather position of each edge's source
    pdst = node_pos[dst]
    ebin = pdst // P                  # destination bin of each edge
    eslot = pdst % P                  # dst slot within bin
    capA, capB = cfg.T_A * P, cfg.T_B * P

    # per-bin edge lists
    idx_sort = np.argsort(ebin, kind="stable")
    ebin_s = ebin[idx_sort]
    bounds = np.searchsorted(ebin_s, np.arange(NBINS + 1))

    idxA = np.zeros((NBINS, capA), dtype=np.int16)
    idxB = np.zeros((NBINS, capB), dtype=np.int16)
    dlA = np.full((NBINS, capA), -1.0, dtype=np.float32)
    dlB = np.full((NBINS, capB), -1.0, dtype=np.float32)
    for b in range(NBINS):
        eids = idx_sort[bounds[b]:bounds[b + 1]]
        ps = psrc[eids]
        sl = eslot[eids]
        a_only = ps < cfg.B_BASE
        b_only = ps >= cfg.WIN
        flex = ~a_only & ~b_only
        nA, nB, nF = a_only.sum(), b_only.sum(), flex.sum()
        assert nA <= capA and nB <= capB, (b, nA, nB)
        flexA = min(nF, capA - nA)
        assert flexA >= 0, (b, nA, capA)
        assert nB + (nF - flexA) <= capB, (b, nA, nB, nF)
        fidx = np.nonzero(flex)[0]
        a_sel = np.concatenate([np.nonzero(a_only)[0], fidx[:flexA]])
        b_sel = np.concatenate([np.nonzero(b_only)[0], fidx[flexA:]])
        idxA[b, :len(a_sel)] = ps[a_sel]
        dlA[b, :len(a_sel)] = sl[a_sel]
        idxB[b, :len(b_sel)] = ps[b_sel] - cfg.B_BASE
        dlB[b, :len(b_sel)] = sl[b_sel]

    return dict(node_pos=node_pos, pos=pos, norm_src=norm_src,
                norm_dst=norm_dst, idxA=idxA, idxB=idxB, dlA=dlA, dlB=dlB)


def _wrap_idx(cfg, idx_bins, Tn):
    """Build the dma_gather index SBUF image for one core & stream:
    [128, total_cols] int16. Within a call, logical index j lives at
    [j%16, j//16]; the 16-row pattern is replicated to all 128 partitions.
    Per-bin images concatenated horizontally are identical to the image of
    any contiguous multi-bin call, so calls may span several bins."""
    cols = []
    for v in idx_bins:                                # one block per bin
        arr = np.zeros((16, len(v) // 16), dtype=np.int16)
        j = np.arange(len(v))
        arr[j % 16, j // 16] = v
        cols.append(arr)
    full = np.concatenate(cols, axis=1)
    return np.tile(full, (8, 1))                      # replicate to 128 parts


def _core_inputs(cfg, core, pre, feat, W1, b1, W2, b2):
    """Build the input-tensor dict for one core."""
    NPC, BINS = cfg.NPC, cfg.BINS
    base = core * NPC
    pslice = pre["pos"][base:base + NPC]              # node id or -1 per slot
    valid = pslice >= 0
    featc = np.zeros((NPC, cfg.F0), dtype=np.float32)
    featc[valid] = feat[pslice[valid]]
    ns = np.ones(NPC, dtype=np.float32)
    nd = np.ones(NPC, dtype=np.float32)
    ns[valid] = pre["norm_src"][pslice[valid]]
    nd[valid] = pre["norm_dst"][pslice[valid]]

    iota_cols = cfg.GROUP * max(cfg.T_A, cfg.T_B) * P
    bsl = slice(core * BINS, (core + 1) * BINS)
    d = {
        "featT": np.ascontiguousarray(featc.T).astype(mybir.dt.np(BF16)),
        "W1": W1.astype(mybir.dt.np(BF16)),
        "W2": W2.astype(mybir.dt.np(BF16)),
        "b1_bc": np.broadcast_to(b1, (P, cfg.F1)).copy().astype(np.float32),
        "b2_bc": np.broadcast_to(b2, (P, cfg.F2)).copy().astype(np.float32),
        "ns_cols": np.ascontiguousarray(ns.reshape(BINS, P).T),
        "nd_cols": np.ascontiguousarray(nd.reshape(BINS, P).T),
        "ndns_cols": np.ascontiguousarray((ns * nd).reshape(BINS, P).T),
        "iota_big": np.broadcast_to(
            np.tile(np.arange(P, dtype=np.float32),
                    iota_cols // P),
            (P, iota_cols)).copy().astype(mybir.dt.np(BF16)),
        "idxA": _wrap_idx(cfg, pre["idxA"][bsl], cfg.T_A),
        "idxB": _wrap_idx(cfg, pre["idxB"][bsl], cfg.T_B),
        "dlA": np.ascontiguousarray(
            pre["dlA"][bsl].reshape(BINS * cfg.T_A, P).T
        ).astype(mybir.dt.np(BF16)),
        "dlB": np.ascontiguousarray(
            pre["dlB"][bsl].reshape(BINS * cfg.T_B, P).T
        ).astype(mybir.dt.np(BF16)),
        "identity": np.eye(P, dtype=np.float32).astype(mybir.dt.np(BF16)),
    }
    return d


# --------------------------------------------------------------------------
# Device program
# --------------------------------------------------------------------------

def build_gcn(tc, outs, ins, cfg, b1_zero, b2_zero, stop_after=99):
    nc = tc.nc
    BINS, GROUP = cfg.BINS, cfg.GROUP
    T_A, T_B = cfg.T_A, cfg.T_B
    F0, F1, F2, F2P = cfg.F0, cfg.F1, cfg.F2, cfg.F2P
    NPC, NPOS = cfg.NPC, cfg.NPOS
    K0, K1 = F0 // P, F1 // P
    out_ap = outs["out"]
    NCH = 4  # allgather chunks per layer
    chunk_at = [round(i * BINS / NCH) for i in range(1, NCH + 1)]
    pairs = [list(range(b, min(b + GROUP, BINS)))
             for b in range(0, BINS, GROUP)]

    import contextlib
    ctx = contextlib.ExitStack()
    with ctx:

        def _bail():
            with tc.tile_pool(name="bailp", bufs=1) as bp:
                zt = bp.tile([P, cfg.F2], F32)
                nc.vector.memset(zt[:], 0.0)
                for b in range(BINS):
                    nc.sync.dma_start(out=out_ap[b * P:(b + 1) * P, :], in_=zt[:])

        constp = ctx.enter_context(tc.tile_pool(name="constp", bufs=1))
        residp = ctx.enter_context(tc.tile_pool(name="residp", bufs=1))
        dramp = ctx.enter_context(tc.tile_pool(name="dramp", bufs=1, space="DRAM"))

        # ---- resident constants ----
        idxA_sb = constp.tile([P, ins["idxA"].shape[1]], I16)
        idxB_sb = constp.tile([P, ins["idxB"].shape[1]], I16)
        dlA_sb = constp.tile([P, BINS * T_A], BF16)
        dlB_sb = constp.tile([P, BINS * T_B], BF16)
        iota_sb = constp.tile([P, GROUP * max(T_A, T_B) * P], BF16)
        ns_sb = constp.tile([P, BINS], F32)
        nd_sb = constp.tile([P, BINS], F32)
        ndns_sb = constp.tile([P, BINS], F32)
        b1_sb = constp.tile([P, F1], F32)
        b2_sb = constp.tile([P, F2], F32)
        ident_sb = constp.tile([P, P], BF16)
        w1_sb = constp.tile([P, K0 * F1], BF16)   # k-chunk c at cols [c*F1, ...)
        w2_sb = constp.tile([P, K1 * F2], BF16)
        for name, t in [("idxA", idxA_sb), ("idxB", idxB_sb), ("dlA", dlA_sb),
                        ("dlB", dlB_sb), ("iota_big", iota_sb),
                        ("ns_cols", ns_sb), ("nd_cols", nd_sb),
                        ("ndns_cols", ndns_sb), ("b1_bc", b1_sb),
                        ("b2_bc", b2_sb), ("identity", ident_sb)]:
            nc.sync.dma_start(out=t[:], in_=ins[name][:])
        for c in range(K0):
            nc.sync.dma_start(out=w1_sb[:, c * F1:(c + 1) * F1],
                              in_=ins["W1"][c * P:(c + 1) * P, :])
        for c in range(K1):
            nc.sync.dma_start(out=w2_sb[:, c * F2:(c + 1) * F2],
                              in_=ins["W2"][c * P:(c + 1) * P, :])

        # residents for layer-2 input
        h1r = residp.tile([P, BINS * F1], BF16)      # relu(h1)*ns, bin-major
        h1rT0 = residp.tile([P, NPC], BF16)          # feat 0:128 transposed
        h1rT1 = residp.tile([P, NPC], BF16)          # feat 128:256

        # DRAM intermediates
        z1_loc = dramp.tile([NPC, F1], BF16)
        z1_full = dramp.tile([NPOS, F1], BF16, addr_space="Shared")
        z2_loc = dramp.tile([NPC, F2P], BF16)
        z2_full = dramp.tile([NPOS, F2P], BF16, addr_space="Shared")
        rg = [list(range(cfg.NC))]

        def allgather(z_loc, z_full):
            nc.gpsimd.collective_compute(
                "AllGather", mybir.AluOpType.bypass, replica_groups=rg,
                ins=[z_loc.opt()], outs=[z_full.opt()])

        # ---- phase 1: z1 = (x @ W1) * ns, allgathered in row chunks ----
        with tc.tile_pool(name="g1", bufs=1) as g1p, \
             tc.tile_pool(name="g1w", bufs=3) as g1w, \
             tc.tile_pool(name="g1ps", bufs=2, space="PSUM") as g1ps:
            featT_sb = g1p.tile([P, K0 * NPC], BF16)
            lo = 0
            for hi in chunk_at:  # chunk-major loads unblock early bins
                for c in range(K0):
                    nc.sync.dma_start(
                        out=featT_sb[:, c * NPC + lo * P:c * NPC + hi * P],
                        in_=ins["featT"][c * P:(c + 1) * P, lo * P:hi * P])
                lo = hi
            for b in range(BINS):
                ps = g1ps.tile([P, F1], F32, space="PSUM", tag="ps")
                for c in range(K0):
                    nc.tensor.matmul(
                        out=ps[:],
                        lhsT=featT_sb[:, c * NPC + b * P: c * NPC + (b + 1) * P],
                        rhs=w1_sb[:, c * F1:(c + 1) * F1],
                        start=(c == 0), stop=(c == K0 - 1))
                zt = g1w.tile([P, F1], BF16, tag="zt")
                nc.vector.tensor_scalar(
                    out=zt[:], in0=ps[:], scalar1=ns_sb[:, b:b + 1],
                    scalar2=None, op0=mybir.AluOpType.mult)
                nc.sync.dma_start(out=z1_loc[b * P:(b + 1) * P, :], in_=zt[:])

        if stop_after < 1:
            _bail()
            return
        allgather(z1_loc, z1_full)

        # Filler matmuls: the PE down-clocks to 1.2 GHz after ~3.4us idle.
        # Keep it busy through the allgather stalls so the aggregation
        # matmuls start (and stay) at 2.4 GHz.
        warmp = ctx.enter_context(tc.tile_pool(name="warmps", bufs=1,
                                               space="PSUM"))

        def warm_pe(n):
            for _ in range(n):
                wp = warmp.tile([P, 512], F32, space="PSUM", tag="wp")
                nc.tensor.matmul(out=wp[:], lhsT=ident_sb[:],
                                 rhs=iota_sb[:, :512], start=True, stop=True)

        warm_pe(60)
        if stop_after < 2:
            _bail()
            return

        qctr = [0]  # swdge queue rotation across all gather calls

        # ---- shared aggregation emitter (pairs of bins per gather) ----
        def aggregate(layer, z_tab, F, FV, epilogue, bin_done=None):
            """layer: 1|2, z_tab: DRAM [NPOS, F], FV: valid cols of gathered
            rows used as matmul rhs. epilogue(b, psum, epp) consumes a bin;
            bin_done(b) is called after each bin's epilogue."""
            with tc.tile_pool(name=f"msg{layer}", bufs=6) as msgp, \
                 tc.tile_pool(name=f"sp{layer}", bufs=4) as sp, \
                 tc.tile_pool(name=f"agg{layer}ps", bufs=4, space="PSUM") as aggps, \
                 tc.tile_pool(name=f"ep{layer}", bufs=3) as epp:
                for bs in pairs:
                    nb, b0 = len(bs), bs[0]
                    mA = msgp.tile([P, GROUP * T_A, F], BF16, tag="mA")
                    mB = msgp.tile([P, GROUP * T_B, F], BF16, tag="mB")
                    nc.gpsimd.dma_gather(
                        out_ap=mA[:, :nb * T_A, :], in_ap=z_tab[0:cfg.WIN, :],
                        idxs_ap=idxA_sb[:, b0 * T_A * 8:(b0 + nb) * T_A * 8],
                        num_idxs=nb * T_A * P, num_idxs_reg=nb * T_A * P,
                        elem_size=F, queue_num=qctr[0] % 4)
                    qctr[0] += 1
                    nc.gpsimd.dma_gather(
                        out_ap=mB[:, :nb * T_B, :],
                        in_ap=z_tab[cfg.B_BASE:NPOS, :],
                        idxs_ap=idxB_sb[:, b0 * T_B * 8:(b0 + nb) * T_B * 8],
                        num_idxs=nb * T_B * P, num_idxs_reg=nb * T_B * P,
                        elem_size=F, queue_num=qctr[0] % 4)
                    qctr[0] += 1
                    sA = sp.tile([P, GROUP * T_A * P], BF16, tag="sA")
                    sB = sp.tile([P, GROUP * T_B * P], BF16, tag="sB")
                    nc.vector.tensor_tensor(
                        out=sA[:, :nb * T_A * P].rearrange(
                            "p (t c) -> p t c", c=P),
                        in0=iota_sb[:, :nb * T_A * P].rearrange(
                            "p (t c) -> p t c", c=P),
                        in1=dlA_sb[:, b0 * T_A:(b0 + nb) * T_A, None]
                            .to_broadcast((P, nb * T_A, P)),
                        op=mybir.AluOpType.is_equal)
                    nc.vector.tensor_tensor(
                        out=sB[:, :nb * T_B * P].rearrange(
                            "p (t c) -> p t c", c=P),
                        in0=iota_sb[:, :nb * T_B * P].rearrange(
                            "p (t c) -> p t c", c=P),
                        in1=dlB_sb[:, b0 * T_B:(b0 + nb) * T_B, None]
                            .to_broadcast((P, nb * T_B, P)),
                        op=mybir.AluOpType.is_equal)
                    for i, b in enumerate(bs):
                        ps = aggps.tile([P, FV], F32, space="PSUM", tag="ps")
                        for t in range(T_A):
                            j = i * T_A + t
                            nc.tensor.matmul(
                                out=ps[:], lhsT=sA[:, j * P:(j + 1) * P],
                                rhs=mA[:, j, :FV],
                                start=(t == 0), stop=False)
                        for t in range(T_B):
                            j = i * T_B + t
                            nc.tensor.matmul(
                                out=ps[:], lhsT=sB[:, j * P:(j + 1) * P],
                                rhs=mB[:, j, :FV],
                                start=False, stop=(t == T_B - 1))
                        epilogue(b, ps, epp)
                        if bin_done is not None:
                            bin_done(b)

        # ---- phase 2: layer-1 aggregation -> h1r, fused z2 GEMM + AG2 ----
        with tc.tile_pool(name="trps", bufs=1, space="PSUM") as trps, \
             tc.tile_pool(name="g2w", bufs=3) as g2w, \
             tc.tile_pool(name="g2ps", bufs=2, space="PSUM") as g2ps:
            def epi1(b, ps, epp):
                hcols = h1r[:, b * F1:(b + 1) * F1]
                if b1_zero:
                    nc.vector.tensor_scalar(
                        out=hcols, in0=ps[:], scalar1=ndns_sb[:, b:b + 1],
                        scalar2=0.0, op0=mybir.AluOpType.mult,
                        op1=mybir.AluOpType.max)
                else:
                    tmp = epp.tile([P, F1], F32, tag="tmp")
                    nc.vector.scalar_tensor_tensor(
                        out=tmp[:], in0=ps[:], scalar=nd_sb[:, b:b + 1],
                        in1=b1_sb[:], op0=mybir.AluOpType.mult,
                        op1=mybir.AluOpType.add)
                    nc.vector.tensor_scalar(
                        out=hcols, in0=tmp[:], scalar1=0.0,
                        scalar2=ns_sb[:, b:b + 1], op0=mybir.AluOpType.max,
                        op1=mybir.AluOpType.mult)
                # transpose the two 128-col halves for the layer-2 GEMM
                for k, hT in ((0, h1rT0), (1, h1rT1)):
                    tp = trps.tile([P, P], BF16, space="PSUM", tag="tp")
                    nc.tensor.transpose(
                        out=tp[:], in_=h1r[:, b * F1 + k * P: b * F1 + (k + 1) * P],
                        identity=ident_sb[:])
                    nc.vector.tensor_copy(out=hT[:, b * P:(b + 1) * P], in_=tp[:])
                # fused phase 3: z2 = h1r @ W2 (padded to F2P cols)
                ps2 = g2ps.tile([P, F2], F32, space="PSUM", tag="ps2")
                nc.tensor.matmul(out=ps2[:], lhsT=h1rT0[:, b * P:(b + 1) * P],
                                 rhs=w2_sb[:, 0:F2], start=True, stop=False)
                nc.tensor.matmul(out=ps2[:], lhsT=h1rT1[:, b * P:(b + 1) * P],
                                 rhs=w2_sb[:, F2:2 * F2], start=False, stop=True)
                z2t = g2w.tile([P, F2P], BF16, tag="z2t")
                nc.vector.tensor_copy(out=z2t[:, :F2], in_=ps2[:])
                nc.vector.memset(z2t[:, F2:], 0.0)
                nc.sync.dma_start(out=z2_loc[b * P:(b + 1) * P, :], in_=z2t[:])

            aggregate(1, z1_full, F1, F1, epi1)

        if stop_after < 3:
            _bail()
            return
        allgather(z2_loc, z2_full)
        warm_pe(40)
        if stop_after < 4:
            _bail()
            return

        # ---- phase 4: layer-2 aggregation -> output ----
        def epi2(b, ps, epp):
            ot = epp.tile([P, F2], F32, tag="ot")
            if b2_zero:
                nc.vector.tensor_scalar(
                    out=ot[:], in0=ps[:, :F2], scalar1=nd_sb[:, b:b + 1],
                    scalar2=None, op0=mybir.AluOpType.mult)
            else:
                nc.vector.scalar_tensor_tensor(
                    out=ot[:], in0=ps[:, :F2], scalar=nd_sb[:, b:b + 1],
                    in1=b2_sb[:], op0=mybir.AluOpType.mult,
                    op1=mybir.AluOpType.add)
            nc.sync.dma_start(out=out_ap[b * P:(b + 1) * P, :], in_=ot[:])

        aggregate(2, z2_full, F2P, F2, epi2)


# --------------------------------------------------------------------------
# Entry point
# --------------------------------------------------------------------------

_cache = {}
_cache_lock = threading.Lock()


def _build_program(cfg, in_specs, b1_zero, b2_zero, stop_after=99, nbody=1):
    nc = bacc.Bacc("TRN2", target_bir_lowering=False, debug=False,
                   num_devices=cfg.NC, num_swdge_queues=4)
    in_aps = {
        name: nc.dram_tensor(name, list(a.shape), mybir.dt.from_np(a.dtype),
                             kind="ExternalInput").ap()
        for name, a in in_specs.items()
    }
    out_aps = {"out": nc.dram_tensor("out", [cfg.NPC, cfg.F2], F32,
                                     kind="ExternalOutput").ap()}
    with tile.TileContext(nc) as tc:
        for _ in range(nbody):
            build_gcn(tc, out_aps, in_aps, cfg, b1_zero, b2_zero,
                      stop_after=stop_after)
    nc.compile()
    return nc


def run_gcn(cfg, feat, src, dst, W1, b1, W2, b2, core_ids=None):
    from concourse import bass_utils

    pre = _preprocess(cfg, np.asarray(src), np.asarray(dst))
    in_maps = [
        _core_inputs(cfg, c, pre, np.asarray(feat, np.float32),
                     np.asarray(W1, np.float32), np.asarray(b1, np.float32),
                     np.asarray(W2, np.float32), np.asarray(b2, np.float32))
        for c in range(cfg.NC)
    ]
    b1_zero = bool(np.all(np.asarray(b1) == 0))
    b2_zero = bool(np.all(np.asarray(b2) == 0))
    stop_after = int(os.environ.get("GCN_STOP_AFTER", "99"))
    key = (id(cfg), b1_zero, b2_zero, stop_after)
    with _cache_lock:
        if key not in _cache:
            _cache[key] = _build_program(cfg, in_maps[0], b1_zero, b2_zero,
                                         stop_after=stop_after)
    nc = _cache[key]

    if core_ids is None:
        core_ids = list(range(cfg.NC))
    res = bass_utils.run_bass_kernel_spmd(
        nc, in_maps, core_ids=core_ids,
        trace=bool(int(os.environ.get("GCN_TRACE", "0"))))
    allout = np.concatenate([r["out"] for r in res.results], axis=0)
    out = allout[pre["node_pos"]].astype(np.float32)
    return out, res


def kernel(feat, src, dst, W1, b1, W2, b2):
    out, _ = run_gcn(CFG_FULL, feat, src, dst, W1, b1, W2, b2)
    return out


# revision 32
# speedup vs baseline: 1.0296x; 1.0098x over previous
"""GCN (2-layer graph conv, DGL norm='both') on 8 Trainium2 NeuronCores.

Strategy
--------
- Nodes are bin-packed into 8*BINS bins of 128 (balanced in-degree sums).
  Each core owns BINS bins (dst-partitioning of the graph).
- Layer GEMMs are transform-first: z = (x * norm_src) @ W, sharded by node
  owner, bf16 inputs / fp32 PSUM accumulate.
- z (bf16) is AllGather'd (in row-chunks, overlapped with the producing
  GEMMs) to every core's HBM; per-edge messages are pulled with dma_gather
  (int16 indices -> two overlapping row-windows A/B of the table so every
  row index fits in int16). Gather calls cover PAIRS of bins (2048 idxs)
  and rotate across the 4 SWDGE queues so descriptor generation runs on
  all four Q7 core-pairs concurrently.
- Segment-sum over edges = selection-matrix matmul: S[p, c] = (dstloc[p]==c)
  built on DVE via is_equal, accumulated per 128-dst bin in PSUM.
- The layer-2 input GEMM (z2 = relu(h1)*ns @ W2) is fused into the layer-1
  aggregation epilogue so the z2 AllGather chunks stream out while later
  bins still aggregate.
- Same edge structure (indices, S data) is reused for both layers.
"""

import math
import os
import threading

import numpy as np

import concourse.bacc as bacc
import concourse.bass as bass
import concourse.mybir as mybir
import concourse.tile as tile

P = 128
F32 = mybir.dt.float32
BF16 = mybir.dt.bfloat16
I16 = mybir.dt.int16


class Cfg:
    def __init__(self, N, E, F0, F1, F2, NC, BINS, T_A, T_B, WIN, GROUP):
        self.N = N            # real node count
        self.E = E            # edge count
        self.F0, self.F1, self.F2 = F0, F1, F2
        self.F2P = 128        # z2 rows padded to 128 cols (gather elem >= 256B)
        self.NC = NC          # cores
        self.BINS = BINS      # dst bins (of 128 nodes) per core
        self.NPC = BINS * P   # padded nodes per core
        self.NPOS = NC * self.NPC
        self.T_A = T_A        # msg tiles per bin from window A
        self.T_B = T_B        # msg tiles per bin from window B
        self.WIN = WIN        # window size (int16 index reach)
        self.B_BASE = max(0, self.NPOS - WIN)
        self.GROUP = GROUP    # bins per gather call
        assert self.NPOS <= WIN + self.B_BASE  # windows cover all rows
        assert self.B_BASE < WIN               # overlap (flex) region exists


CFG_FULL = Cfg(N=50000, E=800000, F0=512, F1=256, F2=64, NC=8,
               BINS=49, T_A=8, T_B=8, WIN=32768,
               GROUP=int(os.environ.get("GCN_GROUP", "1")))


# --------------------------------------------------------------------------
# Host-side graph preprocessing
# --------------------------------------------------------------------------

def _preprocess(cfg, src, dst):
    """Assign nodes to (core, bin, slot) positions and build padded edge
    streams. Returns a dict of per-core numpy arrays plus the node->position
    permutation."""
    N, NC, BINS = cfg.N, cfg.NC, cfg.BINS
    NBINS = NC * BINS
    deg_in = np.bincount(dst, minlength=N).astype(np.int64)
    deg_out = np.bincount(src, minlength=N).astype(np.int64)

    # --- bin-pack nodes by in-degree: balanced sums, <=128 nodes per bin ---
    order = np.argsort(-deg_in, kind="stable")
    bin_sum = np.zeros(NBINS, dtype=np.int64)
    bin_cnt = np.zeros(NBINS, dtype=np.int64)
    bin_nodes = [[] for _ in range(NBINS)]
    import heapq
    heap = [(0, 0, b) for b in range(NBINS)]  # (sum, cnt, bin)
    heapq.heapify(heap)
    for n in order:
        while True:
            s, c, b = heapq.heappop(heap)
            if c < P and s == bin_sum[b] and c == bin_cnt[b]:
                break
        bin_nodes[b].append(n)
        bin_sum[b] += deg_in[n]
        bin_cnt[b] += 1
        if bin_cnt[b] < P:
            heapq.heappush(heap, (int(bin_sum[b]), int(bin_cnt[b]), b))
    capT = (cfg.T_A + cfg.T_B) * P
    assert bin_sum.max() <= capT, (bin_sum.max(), capT)

    # --- positions ---
    pos = np.full(cfg.NPOS, -1, dtype=np.int64)   # position -> node (-1 pad)
    node_pos = np.zeros(N, dtype=np.int64)        # node -> position
    for b in range(NBINS):
        base = b * P
        for s, n in enumerate(bin_nodes[b]):
            pos[base + s] = n
            node_pos[n] = base + s

    # --- norms (match reference._sym_norms) ---
    norm_src = np.where(deg_out > 0, 1.0 / np.sqrt(np.maximum(deg_out, 1)),
                        1.0).astype(np.float32)
    norm_dst = np.where(deg_in > 0, 1.0 / np.sqrt(np.maximum(deg_in, 1)),
                        1.0).astype(np.float32)

    # --- edge streams per bin ---
    psrc = node_pos[src]              # gather position of each edge's source
    pdst = node_pos[dst]
    ebin = pdst // P                  # destination bin of each edge
    eslot = pdst % P                  # dst slot within bin
    capA, capB = cfg.T_A * P, cfg.T_B * P

    # per-bin edge lists
    idx_sort = np.argsort(ebin, kind="stable")
    ebin_s = ebin[idx_sort]
    bounds = np.searchsorted(ebin_s, np.arange(NBINS + 1))

    idxA = np.zeros((NBINS, capA), dtype=np.int16)
    idxB = np.zeros((NBINS, capB), dtype=np.int16)
    dlA = np.full((NBINS, capA), -1.0, dtype=np.float32)
    dlB = np.full((NBINS, capB), -1.0, dtype=np.float32)
    for b in range(NBINS):
        eids = idx_sort[bounds[b]:bounds[b + 1]]
        ps = psrc[eids]
        sl = eslot[eids]
        a_only = ps < cfg.B_BASE
        b_only = ps >= cfg.WIN
        flex = ~a_only & ~b_only
        nA, nB, nF = a_only.sum(), b_only.sum(), flex.sum()
        assert nA <= capA and nB <= capB, (b, nA, nB)
        flexA = min(nF, capA - nA)
        assert flexA >= 0, (b, nA, capA)
        assert nB + (nF - flexA) <= capB, (b, nA, nB, nF)
        fidx = np.nonzero(flex)[0]
        a_sel = np.concatenate([np.nonzero(a_only)[0], fidx[:flexA]])
        b_sel = np.concatenate([np.nonzero(b_only)[0], fidx[flexA:]])
        idxA[b, :len(a_sel)] = ps[a_sel]
        dlA[b, :len(a_sel)] = sl[a_sel]
        idxB[b, :len(b_sel)] = ps[b_sel] - cfg.B_BASE
        dlB[b, :len(b_sel)] = sl[b_sel]

    return dict(node_pos=node_pos, pos=pos, norm_src=norm_src,
                norm_dst=norm_dst, idxA=idxA, idxB=idxB, dlA=dlA, dlB=dlB)


def _wrap_idx(cfg, idx_bins, Tn):
    """Build the dma_gather index SBUF image for one core & stream:
    [128, total_cols] int16. Within a call, logical index j lives at
    [j%16, j//16]; the 16-row pattern is replicated to all 128 partitions.
    Per-bin images concatenated horizontally are identical to the image of
    any contiguous multi-bin call, so calls may span several bins."""
    cols = []
    for v in idx_bins:                                # one block per bin
        arr = np.zeros((16, len(v) // 16), dtype=np.int16)
        j = np.arange(len(v))
        arr[j % 16, j // 16] = v
        cols.append(arr)
    full = np.concatenate(cols, axis=1)
    return np.tile(full, (8, 1))                      # replicate to 128 parts


def _core_inputs(cfg, core, pre, feat, W1, b1, W2, b2):
    """Build the input-tensor dict for one core."""
    NPC, BINS = cfg.NPC, cfg.BINS
    base = core * NPC
    pslice = pre["pos"][base:base + NPC]              # node id or -1 per slot
    valid = pslice >= 0
    featc = np.zeros((NPC, cfg.F0), dtype=np.float32)
    featc[valid] = feat[pslice[valid]]
    ns = np.ones(NPC, dtype=np.float32)
    nd = np.ones(NPC, dtype=np.float32)
    ns[valid] = pre["norm_src"][pslice[valid]]
    nd[valid] = pre["norm_dst"][pslice[valid]]

    iota_cols = cfg.GROUP * max(cfg.T_A, cfg.T_B) * P
    bsl = slice(core * BINS, (core + 1) * BINS)
    d = {
        "featT": np.ascontiguousarray(featc.T).astype(mybir.dt.np(BF16)),
        "W1": W1.astype(mybir.dt.np(BF16)),
        "W2": W2.astype(mybir.dt.np(BF16)),
        "b1_bc": np.broadcast_to(b1, (P, cfg.F1)).copy().astype(np.float32),
        "b2_bc": np.broadcast_to(b2, (P, cfg.F2)).copy().astype(np.float32),
        "ns_cols": np.ascontiguousarray(ns.reshape(BINS, P).T),
        "nd_cols": np.ascontiguousarray(nd.reshape(BINS, P).T),
        "ndns_cols": np.ascontiguousarray((ns * nd).reshape(BINS, P).T),
        "iota_big": np.broadcast_to(
            np.tile(np.arange(P, dtype=np.float32),
                    iota_cols // P),
            (P, iota_cols)).copy().astype(mybir.dt.np(BF16)),
        "idxA": _wrap_idx(cfg, pre["idxA"][bsl], cfg.T_A),
        "idxB": _wrap_idx(cfg, pre["idxB"][bsl], cfg.T_B),
        "dlA": np.ascontiguousarray(
            pre["dlA"][bsl].reshape(BINS * cfg.T_A, P).T
        ).astype(mybir.dt.np(BF16)),
        "dlB": np.ascontiguousarray(
            pre["dlB"][bsl].reshape(BINS * cfg.T_B, P).T
        ).astype(mybir.dt.np(BF16)),
        "identity": np.eye(P, dtype=np.float32).astype(mybir.dt.np(BF16)),
    }
    return d


# --------------------------------------------------------------------------
# Device program
# --------------------------------------------------------------------------

def build_gcn(tc, outs, ins, cfg, b1_zero, b2_zero, stop_after=99):
    nc = tc.nc
    BINS, GROUP = cfg.BINS, cfg.GROUP
    T_A, T_B = cfg.T_A, cfg.T_B
    F0, F1, F2, F2P = cfg.F0, cfg.F1, cfg.F2, cfg.F2P
    NPC, NPOS = cfg.NPC, cfg.NPOS
    K0, K1 = F0 // P, F1 // P
    out_ap = outs["out"]
    NCH = 4  # allgather chunks per layer
    chunk_at = [round(i * BINS / NCH) for i in range(1, NCH + 1)]
    pairs = [list(range(b, min(b + GROUP, BINS)))
             for b in range(0, BINS, GROUP)]

    import contextlib
    ctx = contextlib.ExitStack()
    with ctx:

        def _bail():
            with tc.tile_pool(name="bailp", bufs=1) as bp:
                zt = bp.tile([P, cfg.F2], F32)
                nc.vector.memset(zt[:], 0.0)
                for b in range(BINS):
                    nc.sync.dma_start(out=out_ap[b * P:(b + 1) * P, :], in_=zt[:])

        constp = ctx.enter_context(tc.tile_pool(name="constp", bufs=1))
        residp = ctx.enter_context(tc.tile_pool(name="residp", bufs=1))
        dramp = ctx.enter_context(tc.tile_pool(name="dramp", bufs=1, space="DRAM"))

        # ---- resident constants ----
        idxA_sb = constp.tile([P, ins["idxA"].shape[1]], I16)
        idxB_sb = constp.tile([P, ins["idxB"].shape[1]], I16)
        dlA_sb = constp.tile([P, BINS * T_A], BF16)
        dlB_sb = constp.tile([P, BINS * T_B], BF16)
        iota_sb = constp.tile([P, GROUP * max(T_A, T_B) * P], BF16)
        ns_sb = constp.tile([P, BINS], F32)
        nd_sb = constp.tile([P, BINS], F32)
        ndns_sb = constp.tile([P, BINS], F32)
        b1_sb = constp.tile([P, F1], F32)
        b2_sb = constp.tile([P, F2], F32)
        ident_sb = constp.tile([P, P], BF16)
        w1_sb = constp.tile([P, K0 * F1], BF16)   # k-chunk c at cols [c*F1, ...)
        w2_sb = constp.tile([P, K1 * F2], BF16)
        # W1, ns and the feat chunks go first on the sync DMA FIFO so phase-1
        # matmuls start immediately; the (large) gather-index constants are
        # emitted after the phase-1 loop and land during the allgather stall.
        for c in range(K0):
            nc.sync.dma_start(out=w1_sb[:, c * F1:(c + 1) * F1],
                              in_=ins["W1"][c * P:(c + 1) * P, :])
        nc.sync.dma_start(out=ns_sb[:], in_=ins["ns_cols"][:])

        def load_agg_consts():
            for name, t in [("idxA", idxA_sb), ("idxB", idxB_sb),
                            ("dlA", dlA_sb), ("dlB", dlB_sb),
                            ("iota_big", iota_sb), ("nd_cols", nd_sb),
                            ("ndns_cols", ndns_sb), ("b1_bc", b1_sb),
                            ("b2_bc", b2_sb), ("identity", ident_sb)]:
                nc.sync.dma_start(out=t[:], in_=ins[name][:])
            for c in range(K1):
                nc.sync.dma_start(out=w2_sb[:, c * F2:(c + 1) * F2],
                                  in_=ins["W2"][c * P:(c + 1) * P, :])

        # residents for layer-2 input
        h1r = residp.tile([P, BINS * F1], BF16)      # relu(h1)*ns, bin-major
        h1rT0 = residp.tile([P, NPC], BF16)          # feat 0:128 transposed
        h1rT1 = residp.tile([P, NPC], BF16)          # feat 128:256

        # DRAM intermediates
        z1_loc = dramp.tile([NPC, F1], BF16)
        z1_full = dramp.tile([NPOS, F1], BF16, addr_space="Shared")
        z2_loc = dramp.tile([NPC, F2P], BF16)
        z2_full = dramp.tile([NPOS, F2P], BF16, addr_space="Shared")
        rg = [list(range(cfg.NC))]

        def allgather(z_loc, z_full):
            nc.gpsimd.collective_compute(
                "AllGather", mybir.AluOpType.bypass, replica_groups=rg,
                ins=[z_loc.opt()], outs=[z_full.opt()])

        # ---- phase 1: z1 = (x @ W1) * ns, allgathered in row chunks ----
        with tc.tile_pool(name="g1", bufs=1) as g1p, \
             tc.tile_pool(name="g1w", bufs=3) as g1w, \
             tc.tile_pool(name="g1ps", bufs=2, space="PSUM") as g1ps:
            featT_sb = g1p.tile([P, K0 * NPC], BF16)
            lo = 0
            for hi in chunk_at:  # chunk-major loads unblock early bins
                for c in range(K0):
                    nc.sync.dma_start(
                        out=featT_sb[:, c * NPC + lo * P:c * NPC + hi * P],
                        in_=ins["featT"][c * P:(c + 1) * P, lo * P:hi * P])
                lo = hi
            for b in range(BINS):
                ps = g1ps.tile([P, F1], F32, space="PSUM", tag="ps")
                for c in range(K0):
                    nc.tensor.matmul(
                        out=ps[:],
                        lhsT=featT_sb[:, c * NPC + b * P: c * NPC + (b + 1) * P],
                        rhs=w1_sb[:, c * F1:(c + 1) * F1],
                        start=(c == 0), stop=(c == K0 - 1))
                zt = g1w.tile([P, F1], BF16, tag="zt")
                nc.vector.tensor_scalar(
                    out=zt[:], in0=ps[:], scalar1=ns_sb[:, b:b + 1],
                    scalar2=None, op0=mybir.AluOpType.mult)
                nc.sync.dma_start(out=z1_loc[b * P:(b + 1) * P, :], in_=zt[:])

        load_agg_consts()
        if stop_after < 1:
            _bail()
            return
        allgather(z1_loc, z1_full)

        # Filler matmuls: the PE down-clocks to 1.2 GHz after ~3.4us idle.
        # Keep it busy through the allgather stalls so the aggregation
        # matmuls start (and stay) at 2.4 GHz.
        warmp = ctx.enter_context(tc.tile_pool(name="warmps", bufs=1,
                                               space="PSUM"))

        def warm_pe(n):
            for _ in range(n):
                wp = warmp.tile([P, 512], F32, space="PSUM", tag="wp")
                nc.tensor.matmul(out=wp[:], lhsT=ident_sb[:],
                                 rhs=iota_sb[:, :512], start=True, stop=True)

        warm_pe(150)
        if stop_after < 2:
            _bail()
            return

        qctr = [0]  # swdge queue rotation across all gather calls

        # ---- shared aggregation emitter (pairs of bins per gather) ----
        def aggregate(layer, z_tab, F, FV, epilogue, bin_done=None):
            """layer: 1|2, z_tab: DRAM [NPOS, F], FV: valid cols of gathered
            rows used as matmul rhs. epilogue(b, psum, epp) consumes a bin;
            bin_done(b) is called after each bin's epilogue."""
            with tc.tile_pool(name=f"msg{layer}", bufs=6) as msgp, \
                 tc.tile_pool(name=f"sp{layer}", bufs=4) as sp, \
                 tc.tile_pool(name=f"agg{layer}ps", bufs=4, space="PSUM") as aggps, \
                 tc.tile_pool(name=f"ep{layer}", bufs=3) as epp:
                for bs in pairs:
                    nb, b0 = len(bs), bs[0]
                    mA = msgp.tile([P, GROUP * T_A, F], BF16, tag="mA")
                    mB = msgp.tile([P, GROUP * T_B, F], BF16, tag="mB")
                    nc.gpsimd.dma_gather(
                        out_ap=mA[:, :nb * T_A, :], in_ap=z_tab[0:cfg.WIN, :],
                        idxs_ap=idxA_sb[:, b0 * T_A * 8:(b0 + nb) * T_A * 8],
                        num_idxs=nb * T_A * P, num_idxs_reg=nb * T_A * P,
                        elem_size=F, queue_num=qctr[0] % 4)
                    qctr[0] += 1
                    nc.gpsimd.dma_gather(
                        out_ap=mB[:, :nb * T_B, :],
                        in_ap=z_tab[cfg.B_BASE:NPOS, :],
                        idxs_ap=idxB_sb[:, b0 * T_B * 8:(b0 + nb) * T_B * 8],
                        num_idxs=nb * T_B * P, num_idxs_reg=nb * T_B * P,
                        elem_size=F, queue_num=qctr[0] % 4)
                    qctr[0] += 1
                    sA = sp.tile([P, GROUP * T_A * P], BF16, tag="sA")
                    sB = sp.tile([P, GROUP * T_B * P], BF16, tag="sB")
                    nc.vector.tensor_tensor(
                        out=sA[:, :nb * T_A * P].rearrange(
                            "p (t c) -> p t c", c=P),
                        in0=iota_sb[:, :nb * T_A * P].rearrange(
                            "p (t c) -> p t c", c=P),
                        in1=dlA_sb[:, b0 * T_A:(b0 + nb) * T_A, None]
                            .to_broadcast((P, nb * T_A, P)),
                        op=mybir.AluOpType.is_equal)
                    nc.vector.tensor_tensor(
                        out=sB[:, :nb * T_B * P].rearrange(
                            "p (t c) -> p t c", c=P),
                        in0=iota_sb[:, :nb * T_B * P].rearrange(
                            "p (t c) -> p t c", c=P),
                        in1=dlB_sb[:, b0 * T_B:(b0 + nb) * T_B, None]
                            .to_broadcast((P, nb * T_B, P)),
                        op=mybir.AluOpType.is_equal)
                    for i, b in enumerate(bs):
                        ps = aggps.tile([P, FV], F32, space="PSUM", tag="ps")
                        for t in range(T_A):
                            j = i * T_A + t
                            nc.tensor.matmul(
                                out=ps[:], lhsT=sA[:, j * P:(j + 1) * P],
                                rhs=mA[:, j, :FV],
                                start=(t == 0), stop=False)
                        for t in range(T_B):
                            j = i * T_B + t
                            nc.tensor.matmul(
                                out=ps[:], lhsT=sB[:, j * P:(j + 1) * P],
                                rhs=mB[:, j, :FV],
                                start=False, stop=(t == T_B - 1))
                        epilogue(b, ps, epp)
                        if bin_done is not None:
                            bin_done(b)

        # ---- phase 2: layer-1 aggregation -> h1r, fused z2 GEMM + AG2 ----
        with tc.tile_pool(name="trps", bufs=1, space="PSUM") as trps, \
             tc.tile_pool(name="g2w", bufs=3) as g2w, \
             tc.tile_pool(name="g2ps", bufs=2, space="PSUM") as g2ps:
            def epi1(b, ps, epp):
                hcols = h1r[:, b * F1:(b + 1) * F1]
                if b1_zero:
                    nc.vector.tensor_scalar(
                        out=hcols, in0=ps[:], scalar1=ndns_sb[:, b:b + 1],
                        scalar2=0.0, op0=mybir.AluOpType.mult,
                        op1=mybir.AluOpType.max)
                else:
                    tmp = epp.tile([P, F1], F32, tag="tmp")
                    nc.vector.scalar_tensor_tensor(
                        out=tmp[:], in0=ps[:], scalar=nd_sb[:, b:b + 1],
                        in1=b1_sb[:], op0=mybir.AluOpType.mult,
                        op1=mybir.AluOpType.add)
                    nc.vector.tensor_scalar(
                        out=hcols, in0=tmp[:], scalar1=0.0,
                        scalar2=ns_sb[:, b:b + 1], op0=mybir.AluOpType.max,
                        op1=mybir.AluOpType.mult)
                # transpose the two 128-col halves for the layer-2 GEMM
                for k, hT in ((0, h1rT0), (1, h1rT1)):
                    tp = trps.tile([P, P], BF16, space="PSUM", tag="tp")
                    nc.tensor.transpose(
                        out=tp[:], in_=h1r[:, b * F1 + k * P: b * F1 + (k + 1) * P],
                        identity=ident_sb[:])
                    nc.vector.tensor_copy(out=hT[:, b * P:(b + 1) * P], in_=tp[:])
                # fused phase 3: z2 = h1r @ W2 (padded to F2P cols)
                ps2 = g2ps.tile([P, F2], F32, space="PSUM", tag="ps2")
                nc.tensor.matmul(out=ps2[:], lhsT=h1rT0[:, b * P:(b + 1) * P],
                                 rhs=w2_sb[:, 0:F2], start=True, stop=False)
                nc.tensor.matmul(out=ps2[:], lhsT=h1rT1[:, b * P:(b + 1) * P],
                                 rhs=w2_sb[:, F2:2 * F2], start=False, stop=True)
                z2t = g2w.tile([P, F2P], BF16, tag="z2t")
                nc.vector.tensor_copy(out=z2t[:, :F2], in_=ps2[:])
                nc.vector.memset(z2t[:, F2:], 0.0)
                nc.sync.dma_start(out=z2_loc[b * P:(b + 1) * P, :], in_=z2t[:])

            aggregate(1, z1_full, F1, F1, epi1)

        if stop_after < 3:
            _bail()
            return
        allgather(z2_loc, z2_full)
        warm_pe(40)
        if stop_after < 4:
            _bail()
            return

        # ---- phase 4: layer-2 aggregation -> output ----
        def epi2(b, ps, epp):
            ot = epp.tile([P, F2], F32, tag="ot")
            if b2_zero:
                nc.vector.tensor_scalar(
                    out=ot[:], in0=ps[:, :F2], scalar1=nd_sb[:, b:b + 1],
                    scalar2=None, op0=mybir.AluOpType.mult)
            else:
                nc.vector.scalar_tensor_tensor(
                    out=ot[:], in0=ps[:, :F2], scalar=nd_sb[:, b:b + 1],
                    in1=b2_sb[:], op0=mybir.AluOpType.mult,
                    op1=mybir.AluOpType.add)
            nc.sync.dma_start(out=out_ap[b * P:(b + 1) * P, :], in_=ot[:])

        aggregate(2, z2_full, F2P, F2, epi2)


# --------------------------------------------------------------------------
# Entry point
# --------------------------------------------------------------------------

_cache = {}
_cache_lock = threading.Lock()


def _build_program(cfg, in_specs, b1_zero, b2_zero, stop_after=99, nbody=1):
    nc = bacc.Bacc("TRN2", target_bir_lowering=False, debug=False,
                   num_devices=cfg.NC, num_swdge_queues=4)
    in_aps = {
        name: nc.dram_tensor(name, list(a.shape), mybir.dt.from_np(a.dtype),
                             kind="ExternalInput").ap()
        for name, a in in_specs.items()
    }
    out_aps = {"out": nc.dram_tensor("out", [cfg.NPC, cfg.F2], F32,
                                     kind="ExternalOutput").ap()}
    with tile.TileContext(nc) as tc:
        for _ in range(nbody):
            build_gcn(tc, out_aps, in_aps, cfg, b1_zero, b2_zero,
                      stop_after=stop_after)
    nc.compile()
    return nc


def run_gcn(cfg, feat, src, dst, W1, b1, W2, b2, core_ids=None):
    from concourse import bass_utils

    pre = _preprocess(cfg, np.asarray(src), np.asarray(dst))
    in_maps = [
        _core_inputs(cfg, c, pre, np.asarray(feat, np.float32),
                     np.asarray(W1, np.float32), np.asarray(b1, np.float32),
                     np.asarray(W2, np.float32), np.asarray(b2, np.float32))
        for c in range(cfg.NC)
    ]
    b1_zero = bool(np.all(np.asarray(b1) == 0))
    b2_zero = bool(np.all(np.asarray(b2) == 0))
    stop_after = int(os.environ.get("GCN_STOP_AFTER", "99"))
    key = (id(cfg), b1_zero, b2_zero, stop_after)
    with _cache_lock:
        if key not in _cache:
            _cache[key] = _build_program(cfg, in_maps[0], b1_zero, b2_zero,
                                         stop_after=stop_after)
    nc = _cache[key]

    if core_ids is None:
        core_ids = list(range(cfg.NC))
    res = bass_utils.run_bass_kernel_spmd(
        nc, in_maps, core_ids=core_ids,
        trace=bool(int(os.environ.get("GCN_TRACE", "0"))))
    allout = np.concatenate([r["out"] for r in res.results], axis=0)
    out = allout[pre["node_pos"]].astype(np.float32)
    return out, res


def kernel(feat, src, dst, W1, b1, W2, b2):
    out, _ = run_gcn(CFG_FULL, feat, src, dst, W1, b1, W2, b2)
    return out
